# revision 23
# baseline (speedup 1.0000x reference)
"""GQA causal-attention prefill kernel for 8 Trainium2 NeuronCores.

Reference computation (B=2, S=2048, D=4096, Q=32 q-heads, N=8 kv-heads,
H=128): QKV projection + RoPE + causal GQA attention + O projection.

Sharding: core c handles batch b = c//4 and kv-head pair g = c%4
(kv-heads 2g..2g+1, q-heads 8g..8g+7).  No collectives: each core
computes its partial o-projection (sum over its 8 q-heads) and the host
sums the four partials per batch at gather time (the "all-reduce").

v3 design (vs the v1 three-pass kernel):
  - All matmul operands in bf16 (fp32 PSUM accumulation). Host pre-packs
    every tensor into partition-major layouts so each DMA is contiguous
    per partition.
  - One fully fused phase streaming x ONCE: per s-tile j (512 wide):
    K/V projection -> RoPE -> Q projection (wq streamed per j) -> causal
    attention for all 8 heads -> normalize into SBUF (bf16) -> partial
    o-projection for this s-tile (wo streamed per j).  Projection and
    o-projection matmuls fill TensorE while attention stalls on the exp
    chain; PSUM is balanced 2+2+2+2 banks across the four matmul users.
  - Softmax denominators WITHOUT per-t-tile ones-matmuls: DVE
    accumulates exp-weight tiles into wsum (bf16, 2x DVE mode), GpSimd
    partition_all_reduce gives the broadcast denominator, DVE
    reciprocal+multiply normalizes.  TensorE only does scores + AV in
    the attention inner loop.
  - Diagonal s-blocks compute scores and exp only for the un-masked
    column range (bf16 moving operand keeps 1 cycle/row at any N).
"""

import math
import sys

import numpy as np

for _p in ("/opt/trn_rl_repo", "/root/.axon_site/_ro/trn_rl_repo"):
    if _p not in sys.path:
        sys.path.append(_p)

import concourse.bacc as bacc
import concourse.bass as bass
import concourse.mybir as mybir
import concourse.tile as tile
from concourse import bass_utils

try:
    import ml_dtypes
    BF16_NP = ml_dtypes.bfloat16
except ImportError:  # pragma: no cover
    BF16_NP = None

dt = mybir.dt
F32 = dt.float32
F32R = dt.float32r
BF16 = dt.bfloat16
ADD = mybir.AluOpType.add
MULT = mybir.AluOpType.mult
EXP = mybir.ActivationFunctionType.Exp

FULL_CFG = dict(S=2048, D=4096, QH=8, KH=2, H=128, ST=512)
N_CORES = 8
ROPE_THETA = 10000.0
NEG_BIG = -1.0e30


def build_bass(cfg):
    S, D, QH, KH, H, ST = (cfg["S"], cfg["D"], cfg["QH"], cfg["KH"],
                           cfg["H"], cfg["ST"])
    assert H == 128 and D % 128 == 0 and S % ST == 0 and ST == 512
    DT = D // 128          # d-tiles (contraction tiles for projections)
    HDT = DT // 2          # half of the d-tiles (per xts tile)
    NJ = S // ST           # s-tiles
    TJ = ST // 128         # 128-wide t-tiles per s-tile
    NT = S // 128          # total t-tiles
    G = QH // KH           # GQA group size
    EW = 512               # o-proj output tile width
    NE = D // EW
    scale = 1.0 / math.sqrt(H)

    nc = bacc.Bacc("TRN2", target_bir_lowering=False, debug=False,
                   enable_asserts=False, num_devices=N_CORES)

    xT = nc.dram_tensor("xT", [128, DT, S], BF16, kind="ExternalInput")
    wq = nc.dram_tensor("wq", [QH, 128, DT, H], BF16, kind="ExternalInput")
    wk = nc.dram_tensor("wk", [128, KH, DT, H], BF16, kind="ExternalInput")
    wv = nc.dram_tensor("wv", [128, DT, KH, H], BF16, kind="ExternalInput")
    wo = nc.dram_tensor("wo", [128, QH, D], BF16, kind="ExternalInput")
    cos_d = nc.dram_tensor("cos_t", [128, S], BF16, kind="ExternalInput")
    sin_d = nc.dram_tensor("sin_t", [128, S], BF16, kind="ExternalInput")
    tri_d = nc.dram_tensor("tri_t", [128, 128], BF16, kind="ExternalInput")
    ones_d = nc.dram_tensor("ones_t", [128, 128], BF16, kind="ExternalInput")
    id_d = nc.dram_tensor("id_t", [128, 128], BF16, kind="ExternalInput")
    perm_d = nc.dram_tensor("perm_t", [128, 128], BF16, kind="ExternalInput")
    o_out = nc.dram_tensor("o_out", [S, D], F32, kind="ExternalOutput")

    from contextlib import ExitStack

    with ExitStack() as es:
        tc = es.enter_context(tile.TileContext(nc))
        es.enter_context(
            nc.allow_low_precision(reason="deliberate bf16 pipeline"))
        persist = es.enter_context(tc.tile_pool(name="persist", bufs=1))
        wkvp = es.enter_context(tc.tile_pool(name="wkv", bufs=1))
        xtsp = es.enter_context(tc.tile_pool(name="xts", bufs=3))
        wqp = es.enter_context(tc.tile_pool(name="wqp", bufs=2))
        wop = es.enter_context(tc.tile_pool(name="wop", bufs=3))
        qjp = es.enter_context(tc.tile_pool(name="qj", bufs=16))
        ropep = es.enter_context(tc.tile_pool(name="rope", bufs=2))
        wtp = es.enter_context(tc.tile_pool(name="wt", bufs=8))
        wsp = es.enter_context(tc.tile_pool(name="ws", bufs=2))
        aop = es.enter_context(tc.tile_pool(name="aop", bufs=2))
        ocp = es.enter_context(tc.tile_pool(name="ocp", bufs=2))
        psp = es.enter_context(tc.tile_pool(name="psp", bufs=2, space="PSUM"))
        pss = es.enter_context(tc.tile_pool(name="pss", bufs=2, space="PSUM"))
        pso = es.enter_context(tc.tile_pool(name="pso", bufs=3, space="PSUM"))
        psd = es.enter_context(tc.tile_pool(name="psd", bufs=1, space="PSUM"))
        if True:
            cos_sb = persist.tile([128, S], BF16)
            sin_sb = persist.tile([128, S], BF16)
            tri_sb = persist.tile([128, 128], BF16)
            ones_sb = persist.tile([128, 128], BF16)
            id_sb = persist.tile([128, 128], BF16)
            perm_sb = persist.tile([128, 128], BF16)
            k_sb = persist.tile([128, KH, S], BF16)
            v_sb = persist.tile([128, NT, KH, H], BF16)
            wk_t = wkvp.tile([128, KH, DT, H], BF16)
            wv_t = wkvp.tile([128, DT, KH * H], BF16)

            # Load order matters at startup: the first k-proj group needs
            # only wk[0] + the first half of x; defer everything else.
            for kh in range(KH):
                nc.gpsimd.dma_start(wk_t[:, kh], wk.ap()[:, kh, :, :])
            nc.scalar.dma_start(cos_sb[:], cos_d[:, :])
            nc.scalar.dma_start(sin_sb[:], sin_d[:, :])
            nc.gpsimd.dma_start(wv_t[:],
                                wv.ap().rearrange("p d n h -> p d (n h)"))
            nc.scalar.dma_start(tri_sb[:], tri_d[:, :])
            nc.scalar.dma_start(ones_sb[:], ones_d[:, :])
            nc.scalar.dma_start(id_sb[:], id_d[:, :])
            nc.scalar.dma_start(perm_sb[:], perm_d[:, :])

            def rope(ps_tile, dst_ap, s0):
                """dst = rope(ps_tile) for s-range [s0, s0+ST).

                Rotate-half as a TensorE matmul with a constant
                permutation matrix; sin sign-folded on the host."""
                ta = ropep.tile([128, ST], F32, tag="ta")
                tb = ropep.tile([128, ST], BF16, tag="tb")
                nc.vector.tensor_tensor(ta[:], ps_tile,
                                        cos_sb[:, s0:s0 + ST], MULT)
                nc.vector.tensor_tensor(tb[:], ps_tile,
                                        sin_sb[:, s0:s0 + ST], MULT)
                tbs = pss.tile([128, ST], F32, tag="ps")
                nc.tensor.matmul(tbs[:], perm_sb[:], tb[:],
                                 start=True, stop=True)
                nc.vector.tensor_tensor(dst_ap, ta[:], tbs[:], ADD)

            for j in range(NJ):
                s0 = j * ST
                ssl = slice(s0, s0 + ST)
                with nc.named_scope("proj%d" % j):
                    xa = xtsp.tile([128, HDT, ST], BF16, tag="xts",
                                   name="xa%d" % j)
                    xb = xtsp.tile([128, HDT, ST], BF16, tag="xts",
                                   name="xb%d" % j)
                    hh = HDT // 2
                    nc.sync.dma_start(xa[:, 0:hh], xT.ap()[:, 0:hh, ssl])
                    nc.sync.dma_start(xa[:, hh:HDT], xT.ap()[:, hh:HDT, ssl])
                    nc.sync.dma_start(xb[:, 0:hh], xT.ap()[:, HDT:HDT + hh, ssl])
                    nc.sync.dma_start(xb[:, hh:HDT], xT.ap()[:, HDT + hh:DT, ssl])

                    def xts(di):
                        return (xa[:, di, :] if di < HDT
                                else xb[:, di - HDT, :])

                    # k projection + rope
                    for kh in range(KH):
                        pk = psp.tile([128, ST], F32, tag="pp")
                        for di in range(DT):
                            nc.tensor.matmul(
                                pk[:], wk_t[:, kh, di, :], xts(di),
                                start=(di == 0), stop=(di == DT - 1))
                        rope(pk[:], k_sb[:, kh, ssl], s0)
                    # v projection (stationary = x block, moving = wv)
                    for tl in range(TJ):
                        pv = psp.tile([128, KH * H], F32, tag="pp")
                        for di in range(DT):
                            nc.tensor.matmul(
                                pv[:], xts(di)[:, tl * 128:(tl + 1) * 128],
                                wv_t[:, di, :],
                                start=(di == 0), stop=(di == DT - 1))
                        nc.vector.tensor_copy(v_sb[:, j * TJ + tl, :, :],
                                              pv[:])
                    # q projection + rope (wq streamed per (j, h))
                    qj = []
                    for h in range(QH):
                        wq_t = wqp.tile([128, DT, H], BF16, tag="wq",
                                        name="wq%d_%d" % (j, h))
                        nc.gpsimd.dma_start(wq_t[:], wq.ap()[h])
                        pq = psp.tile([128, ST], F32, tag="pp")
                        for di in range(DT):
                            nc.tensor.matmul(
                                pq[:], wq_t[:, di, :], xts(di),
                                start=(di == 0), stop=(di == DT - 1))
                        qt = qjp.tile([128, ST], BF16, tag="qj",
                                      name="qj%d_%d" % (j, h))
                        rope(pq[:], qt[:], s0)
                        qj.append(qt)

                with nc.named_scope("attn%d" % j):
                    KT = (j + 1) * TJ
                    ao_sb = aop.tile([128, QH, ST], BF16, tag="ao",
                                     name="ao%d" % j)
                    for h in range(QH):
                        kh = h // G
                        po = pso.tile([128, ST], F32, tag="po")
                        wsum = wsp.tile([128, ST], BF16, tag="ws")
                        for kt in range(KT):
                            m = kt - j * TJ
                            lo = m * 128 if m > 0 else 0
                            ps = pss.tile([128, ST], F32, tag="ps")
                            nc.tensor.matmul(
                                ps[:, lo:ST],
                                k_sb[:, kh, kt * 128:(kt + 1) * 128],
                                qj[h][:, lo:ST], start=True,
                                stop=(m < 0), skip_group_check=True)
                            wt = wtp.tile([128, ST], BF16, tag="wt")
                            if m >= 0:
                                # causal mask as a PE matmul: ps += I.T @ tri
                                nc.tensor.matmul(
                                    ps[:, lo:lo + 128], id_sb[:], tri_sb[:],
                                    start=False, stop=True,
                                    skip_group_check=True)
                            nc.scalar.activation(wt[:, lo:ST],
                                                 ps[:, lo:ST],
                                                 EXP, scale=scale)
                            # AV + wsum only on the live column range; the
                            # masked columns of diagonal tiles are never
                            # touched (kt==0 always covers the full width).
                            nc.tensor.matmul(
                                po[:, lo:ST], v_sb[:, kt, kh, :],
                                wt[:, lo:ST],
                                start=(kt == 0), stop=(kt == KT - 1),
                                skip_group_check=True)
                            if kt == 0:
                                nc.vector.tensor_copy(wsum[:], wt[:])
                            else:
                                nc.vector.tensor_tensor(
                                    wsum[:, lo:ST], wsum[:, lo:ST],
                                    wt[:, lo:ST], ADD)
                        # denominator: ones[128,128].T @ wsum gives the
                        # column sums broadcast to every partition; then
                        # 1/den = Exp(-Ln(den)) on ScalarE.
                        den = psd.tile([128, ST], F32, tag="den")
                        nc.tensor.matmul(den[:], ones_sb[:], wsum[:],
                                         start=True, stop=True)
                        t1 = wsp.tile([128, ST], F32, tag="t1")
                        nc.scalar.activation(t1[:], den[:],
                                             mybir.ActivationFunctionType.Ln)
                        rcp = wsp.tile([128, ST], BF16, tag="rcp")
                        nc.scalar.activation(rcp[:], t1[:], EXP, scale=-1.0)
                        nc.vector.tensor_tensor(ao_sb[:, h, :],
                                                po[:], rcp[:], MULT)

                with nc.named_scope("oproj%d" % j):
                    # adjacent-e pairs: each (h, tl) stationary feeds two
                    # consecutive matmuls (one per e of the pair), so the
                    # weight load can be reused by the PE.
                    for ep in range(NE // 2):
                        eA = 2 * ep
                        woeA = wop.tile([128, QH, EW], BF16, tag="woe",
                                        name="woe%d_%d" % (j, eA))
                        woeB = wop.tile([128, QH, EW], BF16, tag="woe",
                                        name="woe%d_%d" % (j, eA + 1))
                        nc.gpsimd.dma_start(
                            woeA[:], wo.ap()[:, :, eA * EW:(eA + 1) * EW])
                        nc.gpsimd.dma_start(
                            woeB[:], wo.ap()[:, :, (eA + 1) * EW:(eA + 2) * EW])
                        for tl in range(TJ):
                            st = j * TJ + tl
                            pcA = pso.tile([128, EW], F32, tag="po")
                            pcB = psd.tile([128, EW], F32, tag="den")
                            for h in range(QH):
                                blk = ao_sb[:, h, tl * 128:(tl + 1) * 128]
                                nc.tensor.matmul(
                                    pcA[:], blk, woeA[:, h, :],
                                    start=(h == 0), stop=(h == QH - 1))
                                nc.tensor.matmul(
                                    pcB[:], blk, woeB[:, h, :],
                                    start=(h == 0), stop=(h == QH - 1))
                            oc2 = ocp.tile([128, 2, EW], F32, tag="oc")
                            nc.vector.tensor_copy(oc2[:, 0, :], pcA[:])
                            nc.vector.tensor_copy(oc2[:, 1, :], pcB[:])
                            nc.sync.dma_start(
                                o_out[st * 128:(st + 1) * 128,
                                      eA * EW:(eA + 2) * EW],
                                oc2[:].rearrange("p a b -> p (a b)"))

    nc.compile()
    return nc


def _perm_matrix():
    P = np.zeros((128, 128), dtype=np.float32)
    P[np.arange(128), (np.arange(128) + 64) % 128] = 1.0
    return P


def make_tables(positions_b, S, H):
    """cos/sin tables in [128, S] layout with the sign fold for the swap
    trick (rows 0:63 -> +sin, 64:127 -> -sin), plus the triangular mask."""
    half = H // 2
    inv_freq = 1.0 / (ROPE_THETA ** (np.arange(half, dtype=np.float64) * 2.0 / H))
    ang = positions_b.astype(np.float64)[None, :] * inv_freq[:, None]  # [half, S]
    cos_h = np.cos(ang)
    sin_h = np.sin(ang)
    cos_t = np.concatenate([cos_h, cos_h], axis=0).astype(np.float32)
    sin_t = np.concatenate([sin_h, -sin_h], axis=0).astype(np.float32)
    idx = np.arange(128)
    tri = np.where(idx[:, None] <= idx[None, :], 0.0, NEG_BIG).astype(np.float32)
    return cos_t, sin_t, tri


def _bf16(a):
    return np.ascontiguousarray(a.astype(BF16_NP))


def _part_major(w):
    """[D, H] -> [128, DT, H] partition-major."""
    D, H = w.shape
    return w.reshape(D // 128, 128, H).transpose(1, 0, 2)


def make_in_maps(x, positions, Wq, Wk, Wv, Wo, cfg):
    """Shard the full inputs into the 8 per-core input maps."""
    QH, KH = cfg["QH"], cfg["KH"]
    S, H = cfg["S"], cfg["H"]
    D = x.shape[2]
    DT = D // 128
    B = x.shape[0]
    groups = N_CORES // B
    tables = [make_tables(np.asarray(positions[b]), S, H) for b in range(B)]
    in_maps = []
    for c in range(N_CORES):
        b, g = divmod(c, groups)
        cos_t, sin_t, tri = tables[b]
        xb = np.asarray(x[b])                       # [S, D]
        xT = xb.T.reshape(DT, 128, S).transpose(1, 0, 2)   # [128, DT, S]
        wq_l = np.stack([_part_major(Wq[g * QH + h]) for h in range(QH)])
        wk_l = np.stack([_part_major(Wk[g * KH + n]) for n in range(KH)],
                        axis=1)                     # [128, KH, DT, H]
        wv_l = np.stack([_part_major(Wv[g * KH + n]) for n in range(KH)],
                        axis=2)                     # [128, DT, KH, H]
        wo_l = np.asarray(Wo[g * QH:(g + 1) * QH]).transpose(1, 0, 2)  # [H, QH, D]
        in_maps.append({
            "xT": _bf16(xT),
            "wq": _bf16(wq_l),
            "wk": _bf16(wk_l),
            "wv": _bf16(wv_l),
            "wo": _bf16(wo_l),
            "cos_t": _bf16(cos_t),
            "sin_t": _bf16(sin_t),
            "tri_t": _bf16(tri),
            "ones_t": _bf16(np.ones((128, 128), dtype=np.float32)),
            "id_t": _bf16(np.eye(128, dtype=np.float32)),
            "perm_t": _bf16(_perm_matrix()),
        })
    return in_maps


_NC_CACHE = {}


def _get_nc(cfg_key=None):
    cfg = FULL_CFG if cfg_key is None else cfg_key
    key = tuple(sorted(cfg.items()))
    if key not in _NC_CACHE:
        _NC_CACHE[key] = build_bass(cfg)
    return _NC_CACHE[key]


def run(x, positions, Wq, Wk, Wv, Wo, trace=False, trace_kwargs=None):
    cfg = FULL_CFG
    nc = _get_nc(cfg)
    in_maps = make_in_maps(np.asarray(x), np.asarray(positions),
                           np.asarray(Wq), np.asarray(Wk), np.asarray(Wv),
                           np.asarray(Wo), cfg)
    res = bass_utils.run_bass_kernel_spmd(
        nc, in_maps, list(range(N_CORES)), trace=trace,
        **(trace_kwargs or {}))
    B = np.asarray(x).shape[0]
    groups = N_CORES // B
    outs = []
    for b in range(B):
        acc = res.results[b * groups]["o_out"].astype(np.float64)
        for g in range(1, groups):
            acc += res.results[b * groups + g]["o_out"]
        outs.append(acc.astype(np.float32))
    return np.stack(outs, axis=0), res


def kernel(x, positions, Wq, Wk, Wv, Wo):
    out, _ = run(x, positions, Wq, Wk, Wv, Wo, trace=False)
    return out
